# revision 41
# baseline (speedup 1.0000x reference)
"""Cross-attention (GQA) Trainium2 Bass kernel — pipelined v2.

Problem: B=2, Tq=Tkv=2048, D_MODEL=1024, 16 query heads / 4 kv heads,
head_dim=64.  Sharded over 8 NeuronCores as batch(2) x kv-group(4); each
core computes 4 query heads + its single kv head and a partial output
projection (Wo row-split by head group); partials are summed on host.

Dataflow (feature dim on SBUF partitions end-to-end, no big transposes):

  A: qT[e,t] = WqT.T @ xqT,  kvT = WkvT.T @ xcT   (weights stationary)
     v[tk,dv] via PE-transpose of vT tiles; vp=[v|1], vp2=[1|v]
  B: per (blk,e) section, unit t: pb[128,1024] = two K=64 row-group
     matmuls (h_even rows 0-63 -> cols 0:512, h_odd rows 64-127 ->
     cols 512:1024), concurrent in the PE array.
  C: pt = exp(pb/8) one ScalarE instruction per unit (FD=1024).
  D: pd_h[128,512] += vp_t.T @ pt_half; ones-columns give the softmax
     denominators in the complementary 64 partitions.
  E: yT += WoT.T @ (pd*recip(den)), row-split by head pair.

The whole BCD stream is software-pipelined: the PE emission order is
B(t), D(t-1) so matmuls never wait on the ScalarE exp of the same unit;
projection/output-projection matmuls are fed as "fill" work into the
PE slack inside each section.  ScalarE (the 1 elem/cycle/lane exp
bottleneck, ~143us) paces the kernel; the PE stays dense and HAM-warm.
"""

import os
import sys
from collections import deque

import numpy as np

for _p in ("/opt/trn_rl_repo",):
    if _p not in sys.path and os.path.isdir(_p):
        sys.path.insert(0, _p)

import concourse.bass as bass
import concourse.bacc as bacc
import concourse.mybir as mybir
from concourse.tile import TileContext

# ---------------------------------------------------------------- problem dims
B = 2
TQ = 2048
TKV = 2048
D_MODEL = 1024
N_HEADS = 16
N_KV_HEADS = 4
HEAD_DIM = 64
N_CORES = 8
GROUPS = N_KV_HEADS  # kv groups = 4
HEADS_PER_DEV = N_HEADS // GROUPS  # 4
DQ = HEADS_PER_DEV * HEAD_DIM  # 256
DKV = 2 * HEAD_DIM  # 128 (k rows + v rows stacked)
SCALE = 1.0 / float(np.sqrt(HEAD_DIM))

P = 128
FREE = 512  # matmul moving-operand chunk / tq block width
BLK = 512
NBLK = TQ // BLK  # 4 tq blocks
DT = D_MODEL // P  # 8 d-tiles
ET = DQ // P  # 2 e-tiles (query head pairs)
NCH = TQ // FREE  # 4 x chunks of 512
NTK = TKV // P  # 16 tk tiles
MT = D_MODEL // P  # 8 output m-tiles

F32 = mybir.dt.float32
F16 = mybir.dt.float16


def build_bass():
    nc = bacc.Bacc()

    xq = nc.declare_dram_parameter("xqT", [D_MODEL, TQ], F16, isOutput=False)
    xc = nc.declare_dram_parameter("xcT", [D_MODEL, TKV], F16, isOutput=False)
    wq = nc.declare_dram_parameter("wqT", [D_MODEL, DQ], F16, isOutput=False)
    wkv = nc.declare_dram_parameter("wkvT", [D_MODEL, DKV], F16, isOutput=False)
    wo = nc.declare_dram_parameter("woT", [DQ, D_MODEL], F16, isOutput=False)
    cid = nc.declare_dram_parameter("cid", [P, P], F16, isOutput=False)
    yt = nc.declare_dram_parameter("yT", [D_MODEL, TQ], F16, isOutput=True)

    with TileContext(nc) as tc:
        with (
            tc.tile_pool(name="consts", bufs=1) as consts,
            tc.tile_pool(name="xch", bufs=2) as xpool,
            tc.tile_pool(name="pt", bufs=4) as ptpool,
            tc.tile_pool(name="rec", bufs=2) as recpool,
            tc.tile_pool(name="yout", bufs=3) as ypool,
            tc.tile_pool(name="psS", bufs=2, space="PSUM") as psS,
            tc.tile_pool(name="psD", bufs=1, space="PSUM") as psD,
            tc.tile_pool(name="psA", bufs=2, space="PSUM") as psA,
        ):
            # ---------------- constants / persistent tiles
            # DMA priority order: the lead-in critical path is
            # xc0+wkv (kv proj) then xq0+wq (q proj) -> first B matmul.
            qt = consts.tile([P, ET, TQ], F16, tag="qt")  # head pair per e
            kv = consts.tile([P, TKV], F16, tag="kv")  # rows 0-63 kT, 64-127 vT
            k2 = consts.tile([P, TKV], F16, tag="k2")  # rows 64-127 = kT copy
            vp = consts.tile([P, NTK, P], F16, tag="vp")  # [v | ones]
            vp2 = consts.tile([P, NTK, P], F16, tag="vp2")  # [ones | v]
            outs = consts.tile([P, ET, TQ], F16, tag="outs")  # normalized outT

            # input chunk dmas (xpool rotates 2 bufs per tag)
            def dma_xc(c):
                cs = slice(c * FREE, (c + 1) * FREE)
                t = xpool.tile([P, DT, FREE], F16, tag="xc", name=f"xc{c}", bufs=3)
                nc.sync.dma_start(t, xc.rearrange("(i p) t -> p i t", p=P)[:, :, cs])
                return t

            def dma_xq(c):
                cs = slice(c * FREE, (c + 1) * FREE)
                t = xpool.tile([P, DT, FREE], F16, tag="xq", name=f"xq{c}", bufs=3)
                nc.sync.dma_start(t, xq.rearrange("(i p) t -> p i t", p=P)[:, :, cs])
                return t

            ident = consts.tile([P, P], F16, tag="ident")
            nc.sync.dma_start(ident, cid[:])
            xc_t = [None] * NCH
            xq_t = [None] * NCH
            xc_t[0] = dma_xc(0)
            wkv_sb = consts.tile([P, DT, DKV], F16, tag="wkv")
            nc.sync.dma_start(wkv_sb, wkv.rearrange("(i p) e -> p i e", p=P))
            xq_t[0] = dma_xq(0)
            wq_sb = consts.tile([P, DT, DQ], F16, tag="wq")
            nc.sync.dma_start(wq_sb, wq.rearrange("(i p) e -> p i e", p=P))
            wo_sb = consts.tile([P, ET, D_MODEL], F16, tag="wo")

            nc.vector.memset(vp, 1.0)
            nc.vector.memset(vp2, 1.0)

            # Warm-up while input DMAs stream: ~3.5us of dummy matmuls gets
            # the PE HAM clock-gate to 8/8 (2.4GHz) before the projections;
            # a tiny exp pulls the ScalarE ACT table load off the hot path.
            dum = consts.tile([P, 8], F16, tag="dum")
            nc.scalar.activation(
                dum, ident[:, :8], mybir.ActivationFunctionType.Exp, bias=0.0, scale=1.0
            )
            warm = psA.tile([P, P], F32, tag="pa", name="warm")
            for i in range(30):
                nc.tensor.matmul(warm, ident, ident, start=(i == 0), stop=(i == 29))

            # ---------------- fill-work machinery (PE slack consumers)
            fills = deque()

            def pop_fill(n=1):
                for _ in range(n):
                    if not fills:
                        return
                    fills.popleft()()

            # D matmuls for one pipelined unit (two heads, K=128, N=512)
            def emit_d(pd0, pd1, pt, t):
                nc.tensor.matmul(
                    pd0, vp[:, t, :], pt[:, :BLK],
                    start=(t == 0), stop=(t == NTK - 1), skip_group_check=True,
                )
                nc.tensor.matmul(
                    pd1, vp2[:, t, :], pt[:, BLK:],
                    start=(t == 0), stop=(t == NTK - 1), skip_group_check=True,
                )

            # kv projection chunk: 8 K-tiles -> kv[:, cs]; k2 copy; transposes
            def kv_chunk_pieces(c, get_xc):
                cs = slice(c * FREE, (c + 1) * FREE)
                st = {}

                def pk(i0):
                    def p():
                        if i0 == 0:
                            st["pkv"] = psA.tile([P, FREE], F32, tag="pa", name="pkv")
                        for i in range(i0, i0 + 2):
                            nc.tensor.matmul(
                                st["pkv"], wkv_sb[:, i, :], get_xc()[:, i, :],
                                start=(i == 0), stop=(i == DT - 1),
                            )
                        if i0 == DT - 2:
                            nc.vector.tensor_copy(kv[:, cs], st["pkv"])
                            nc.sync.dma_start(k2[HEAD_DIM:, cs], kv[:HEAD_DIM, cs])

                    return p

                def p3():
                    # transpose the 4 v tiles of this chunk, batch-copy to vp/vp2
                    pvb = psA.tile([P, 4 * HEAD_DIM], F16, tag="pa", name="pvb")
                    for k in range(4):
                        ts_ = slice((4 * c + k) * P, (4 * c + k + 1) * P)
                        nc.tensor.transpose(
                            pvb[:, k * HEAD_DIM : (k + 1) * HEAD_DIM],
                            kv[HEAD_DIM:, ts_],
                            ident[HEAD_DIM:, HEAD_DIM:],
                        )
                    src = pvb.rearrange("p (k d) -> p k d", k=4)
                    nc.vector.tensor_copy(vp[:, 4 * c : 4 * c + 4, :HEAD_DIM], src)
                    nc.vector.tensor_copy(vp2[:, 4 * c : 4 * c + 4, HEAD_DIM:], src)

                return [pk(0), pk(2), pk(4), pk(6), p3]

            # q projection chunk (one e): 8 K-tiles -> qt[:, e, cs]
            def q_chunk_pieces(c, e, get_xq):
                cs = slice(c * FREE, (c + 1) * FREE)
                st = {}

                def pq(i0):
                    def p():
                        if i0 == 0:
                            st["pq"] = psA.tile([P, FREE], F32, tag="pa", name="pq")
                        for i in range(i0, i0 + 2):
                            nc.tensor.matmul(
                                st["pq"], wq_sb[:, i, e * P : (e + 1) * P],
                                get_xq()[:, i, :],
                                start=(i == 0), stop=(i == DT - 1),
                            )
                        if i0 == DT - 2:
                            nc.scalar.copy(qt[:, e, cs], st["pq"])

                    return p

                return [pq(0), pq(2), pq(4), pq(6)]

            # output-projection piece for one m-tile of one tq block
            def e_piece(blk, m):
                bs = slice(blk * BLK, (blk + 1) * BLK)
                ms = slice(m * P, (m + 1) * P)

                def p():
                    py = psA.tile([P, FREE], F32, tag="pa", name="py")
                    for ee in range(ET):
                        nc.tensor.matmul(
                            py, wo_sb[:, ee, ms], outs[:, ee, bs],
                            start=(ee == 0), stop=(ee == ET - 1),
                        )
                    yo = ypool.tile([P, FREE], F16, tag="yo", name="yo")
                    nc.scalar.copy(yo, py)
                    nc.sync.dma_start(yt[ms, bs], yo)

                return p

            # ---------------- lead-in: minimum inline work before section 0:
            # kv c0 projection + q c0 (e=0 only); everything else is fills.
            kc0 = kv_chunk_pieces(0, lambda: xc_t[0])
            for piece in kc0[:4]:
                piece()
            for piece in q_chunk_pieces(0, 0, lambda: xq_t[0]):
                piece()

            # Fill order is a DEADLINE order: section-0 units consume 2
            # pieces/unit and a piece's writes are only visible to LATER-
            # emitted readers (Tile deps follow emission order).  kv chunk c
            # must be fully emitted before B(t=4c); vp transposes for chunk c
            # before D(4c), which lags B by 2 units.
            xc_t[1] = dma_xc(1)
            kc1 = kv_chunk_pieces(1, lambda: xc_t[1])
            for piece in kc1[:4]:
                piece()
            fills.append(lambda: xq_t.__setitem__(1, dma_xq(1)))
            fills.append(kc0[4])  # v transposes for chunk 0 (D(0) is unit 2)
            fills.append(kc1[4])
            # xc2/xc3 dmas go AFTER kv c1's k2 broadcast so they don't block
            # it in the single sync-DMA queue
            fills.append(lambda: xc_t.__setitem__(2, dma_xc(2)))
            fills.append(lambda: xc_t.__setitem__(3, dma_xc(3)))
            fills.extend(kv_chunk_pieces(2, lambda: xc_t[2]))
            fills.extend(kv_chunk_pieces(3, lambda: xc_t[3]))
            fills.extend(q_chunk_pieces(0, 1, lambda: xq_t[0]))
            fills.append(
                lambda: nc.sync.dma_start(wo_sb, wo.rearrange("(i p) m -> p i m", p=P))
            )
            for e in range(ET):
                fills.extend(q_chunk_pieces(1, e, lambda: xq_t[1]))

            # ---------------- BCD sections
            for sec, (blk, e) in enumerate(
                (blk, e) for blk in range(NBLK) for e in range(ET)
            ):
                bs = slice(blk * BLK, (blk + 1) * BLK)
                pd0 = psD.tile([P, BLK], F32, tag="pd0", name="pd0")
                pd1 = psD.tile([P, BLK], F32, tag="pd1", name="pd1")
                pending = deque()  # D lags B by 2 units so exp sems are settled
                for t in range(NTK):
                    ts_ = slice(t * P, (t + 1) * P)
                    pb = psS.tile([P, 2 * BLK], F32, tag="pb", name="pb")
                    # B: two K=64 row-group matmuls, concurrent in the array
                    nc.tensor.matmul(pb[:, :BLK], kv[:HEAD_DIM, ts_], qt[:HEAD_DIM, e, bs])
                    nc.tensor.matmul(pb[:, BLK:], k2[HEAD_DIM:, ts_], qt[HEAD_DIM:, e, bs])
                    if len(pending) >= 2:
                        emit_d(*pending.popleft())
                    pt = ptpool.tile([P, 2 * BLK], F16, tag="pt", name="pt")
                    nc.scalar.activation(
                        pt, pb, mybir.ActivationFunctionType.Exp, bias=0.0, scale=SCALE
                    )
                    if sec == 0:
                        pop_fill(2)
                    elif t >= 2:
                        pop_fill(1)
                    pending.append((pd0, pd1, pt, t))
                while pending:
                    emit_d(*pending.popleft())

                # normalize: spill pd fast (frees PSUM for the next section),
                # then recip(den) -> broadcast -> outs = out * rec off-path.
                # The last section skips the spill (no successor needs pd).
                last = sec == NBLK * ET - 1
                if last:
                    raw0, raw1 = pd0, pd1
                else:
                    raw0 = recpool.tile([P, BLK], F32, tag="raw0", name="raw0")
                    raw1 = recpool.tile([P, BLK], F32, tag="raw1", name="raw1")
                    nc.scalar.copy(raw0, pd0)
                    nc.scalar.copy(raw1, pd1)
                rec0 = recpool.tile([P, BLK], F32, tag="rec0", name="rec0")
                rec1 = recpool.tile([P, BLK], F32, tag="rec1", name="rec1")
                nc.vector.reciprocal(rec0[HEAD_DIM:, :], raw0[HEAD_DIM:, :])
                nc.sync.dma_start(rec0[:HEAD_DIM, :], rec0[HEAD_DIM:, :])
                nc.vector.reciprocal(rec1[:HEAD_DIM, :], raw1[:HEAD_DIM, :])
                nc.vector.tensor_mul(
                    outs[:HEAD_DIM, e, bs], raw0[:HEAD_DIM, :], rec0[:HEAD_DIM, :]
                )
                nc.sync.dma_start(rec1[HEAD_DIM:, :], rec1[:HEAD_DIM, :])
                nc.vector.tensor_mul(
                    outs[HEAD_DIM:, e, bs], raw1[HEAD_DIM:, :], rec1[HEAD_DIM:, :]
                )

                # queue follow-on work.  x dmas issue a full section before
                # their consuming projection pieces; E pieces go LAST so they
                # pop only after the producing normalize has finished (an
                # early E piece stalls the in-order PE on outs and lets the
                # HAM clock-gate go cold).
                if sec == 0:
                    fills.append(lambda: xq_t.__setitem__(2, dma_xq(2)))
                if sec == 1:
                    for ee in range(ET):
                        fills.extend(q_chunk_pieces(2, ee, lambda: xq_t[2]))
                if sec == 2:
                    fills.append(lambda: xq_t.__setitem__(3, dma_xq(3)))
                if sec == 3:
                    for ee in range(ET):
                        fills.extend(q_chunk_pieces(3, ee, lambda: xq_t[3]))
                # E(b) pieces are deferred a FULL section past b's last
                # normalize so they never stall the in-order PE on outs
                if sec in (2, 4, 6):
                    for m in range(MT):
                        fills.append(e_piece(sec // 2 - 1, m))
                if sec == NBLK * ET - 1:
                    for m in range(MT):
                        fills.append(e_piece(blk, m))

            # tail: keep the PE clocked while the final normalize chain runs
            # on the DVE, then drain the last block's output projection
            wtail = psA.tile([P, P], F32, tag="pa", name="wtail")
            for i in range(30):
                nc.tensor.matmul(wtail, ident, ident, start=(i == 0), stop=(i == 29))
            while fills:
                pop_fill()

    nc.finalize()
    return nc


_NC_CACHE = None


def _get_nc():
    global _NC_CACHE
    if _NC_CACHE is None:
        _NC_CACHE = build_bass()
    return _NC_CACHE


def _cid():
    return np.eye(P, dtype=np.float16)


def shard_inputs(query, context, Wq, Wk, Wv, Wo):
    """host-side sharding: 8 cores = batch(2) x kv-group(4)"""
    in_maps = []
    xqT = [np.ascontiguousarray(query[b].T).astype(np.float16) for b in range(B)]
    xcT = [np.ascontiguousarray(context[b].T).astype(np.float16) for b in range(B)]
    for core in range(N_CORES):
        b, g = divmod(core, GROUPS)
        wqT = np.ascontiguousarray(Wq[g * DQ : (g + 1) * DQ, :].T).astype(np.float16)
        wkvT = np.ascontiguousarray(
            np.concatenate(
                [
                    Wk[g * HEAD_DIM : (g + 1) * HEAD_DIM, :],
                    Wv[g * HEAD_DIM : (g + 1) * HEAD_DIM, :],
                ],
                axis=0,
            ).T
        ).astype(np.float16)
        woT = np.ascontiguousarray(Wo[:, g * DQ : (g + 1) * DQ].T).astype(np.float16)
        in_maps.append(
            {
                "xqT": xqT[b],
                "xcT": xcT[b],
                "wqT": wqT,
                "wkvT": wkvT,
                "woT": woT,
                "cid": _cid(),
            }
        )
    return in_maps


def kernel(query, context, Wq, Wk, Wv, Wo, _want_profile=False):
    from concourse.bass_utils import run_bass_kernel_spmd

    nc = _get_nc()
    in_maps = shard_inputs(query, context, Wq, Wk, Wv, Wo)
    res = run_bass_kernel_spmd(
        nc, in_maps, core_ids=list(range(N_CORES)), trace=_want_profile
    )
    out = np.zeros((B, TQ, D_MODEL), dtype=np.float32)
    for core in range(N_CORES):
        b = core // GROUPS
        out[b] += res.results[core]["yT"].T.astype(np.float32)
    if _want_profile:
        return out, res
    return out
